# revision 3
# baseline (speedup 1.0000x reference)
"""LogSparseAttention Trainium2 kernel.

B,L,H,E = 2,2048,8,64 ; S,D = 2048,64 ; fp32 in/out.
Shard B*H = 16 (b,h) pairs across 8 cores, 2 pairs/core.

Per (b,h): scores^T[j,i] = K[j]·Q[i] computed only on a sparse set of
"stripe" windows per 128-row j-chunk:
  band  : i in [128c, 128c+272)          covers delta = i-j in {0..12,14,18,26,42,74,138} (+rows i<22 full causal)
  far-d : i in [128c+d, 128c+d+128)      covers delta == d for d in {266, 522, 1034}
exp on ScalarE (no max-subtraction needed: |score*scale| <= ~6), 0/1 bf16
mask multiply on VectorE, then PV matmuls (V augmented with a ones column
so row 64 of O^T accumulates Z) accumulate into a PSUM O^T [65, 2048].
Epilogue: copy->SBUF, PE-transpose per 128-col tile, multiply by 1/Z,
DMA out. Matmuls in bf16 (inputs pre-cast on host).
"""

import math

import ml_dtypes
import numpy as np

B, L, H, E = 2, 2048, 8, 64
S, D = 2048, 64
NC_CORES = 8
PAIRS_PER_CORE = 2
CH = L // 128  # 16 chunks
SCALE = 1.0 / math.sqrt(E)

WB = 272                      # band window width
FARS = (266, 522, 1034)       # far diagonals, 128-wide windows each
WIN_A = WB + 128              # psA: band | far266
WIN_B = 256                   # psB: far522 | far1034
WTOT = WIN_A + WIN_B          # 656 mask row width per chunk
QTW = 3104                    # QT padded width >= 128*15 + 1034 + 128 = 3082

BAND_SET = frozenset(list(range(0, 13)) + [14, 18, 26, 42, 74, 138])


# ---------------------------------------------------------------- host masks
def _full_mask() -> np.ndarray:
    """Replica of the reference log-sparse mask [L, S] (0/1 float32)."""
    log_l = math.ceil(math.log2(L))
    m = np.zeros((L, S), dtype=np.float32)
    for index in range(L):
        row = np.zeros(S, dtype=np.float32)
        if (S // L) * 2 * log_l > index:
            row[: index + 1] = 1.0
        else:
            idx = index
            while idx >= 0:
                if idx - log_l + 1 < 0:
                    row[:idx] = 1.0
                    break
                row[idx - log_l + 1 : idx + 1] = 1.0
                for i in range(log_l):
                    new_index = idx - log_l + 1 - 2**i
                    if idx - new_index <= L and new_index >= 0:
                        row[new_index] = 1.0
                idx -= L
        m[index] = row
    return m


def _window_masks():
    """Per-chunk [128, WTOT] 0/1 masks in S^T orientation, deduplicated.

    Returns (masks_np [128, ndist*WTOT] bf16, idx_per_chunk list[int]).
    Also asserts the windows exactly tile the reference mask.
    """
    mf = _full_mask()
    scatter = np.zeros_like(mf)
    per_c = []
    for c in range(CH):
        m = np.zeros((128, WTOT), dtype=np.float32)
        j0 = 128 * c
        # band: cols [0, WB): i = j0 + f, j = j0 + p
        for p in range(128):
            j = j0 + p
            for f in range(WB):
                i = j0 + f
                if i >= L or j > i:
                    continue
                d = i - j
                if (i < 22 and j <= i) or (i >= 22 and d in BAND_SET):
                    if mf[i, j] != 1.0:
                        raise AssertionError(f"band mask mismatch i={i} j={j}")
                    m[p, f] = 1.0
                    scatter[i, j] += 1.0
        # far windows
        for wi, dd in enumerate(FARS):
            off = WB + 128 * wi
            for p in range(128):
                j = j0 + p
                i = j + dd
                if i >= L:
                    continue
                if mf[i, j] != 1.0:
                    raise AssertionError(f"far mask mismatch i={i} j={j}")
                m[p, off + p] = 1.0
                scatter[i, j] += 1.0
        per_c.append(m)
    if not np.array_equal(scatter, mf):
        bad = np.argwhere(scatter != mf)
        raise AssertionError(f"window masks do not tile reference mask: {bad[:5]}")
    # dedupe
    distinct, idx_per_chunk = [], []
    seen = {}
    for m in per_c:
        key = m.tobytes()
        if key not in seen:
            seen[key] = len(distinct)
            distinct.append(m)
        idx_per_chunk.append(seen[key])
    masks_np = np.concatenate(distinct, axis=1).astype(ml_dtypes.bfloat16)
    return masks_np, idx_per_chunk


_MASKS_NP, _MASK_IDX = _window_masks()
_NDIST = _MASKS_NP.shape[1] // WTOT


# ---------------------------------------------------------------- PV pieces
def _pv_pieces(c):
    """PV matmul pieces for chunk c.

    Each: (tile_id 'A'|'B', src_off, dst_start, width, stop).
    dst ranges clipped to [0, L), split at 512-col PSUM bank bounds; the
    band's first 128 cols form their own piece with stop=True (last writer
    of O^T cols [128c, 128c+128)).
    """
    pieces = []

    def add(tile_id, src_off, dst_start, width, stop):
        if dst_start >= L:
            return
        width = min(width, L - dst_start)
        if width <= 0:
            return
        a = dst_start
        while a < dst_start + width:
            b = min(dst_start + width, (a // 512 + 1) * 512)
            pieces.append((tile_id, src_off + (a - dst_start), a, b - a, stop))
            a = b

    j0 = 128 * c
    add("A", 0, j0, 128, True)            # band head (stop)
    add("A", 128, j0 + 128, WB - 128, False)  # band tail
    add("A", WB, j0 + FARS[0], 128, False)
    add("B", 0, j0 + FARS[1], 128, False)
    add("B", 128, j0 + FARS[2], 128, False)
    return pieces


# ---------------------------------------------------------------- bass build
_CACHE = {}


def _build_nc():
    import concourse.bacc as bacc
    import concourse.bass as bass
    import concourse.mybir as mybir
    import concourse.tile as tile

    f32 = mybir.dt.float32
    bf16 = mybir.dt.bfloat16
    AF = mybir.ActivationFunctionType

    nc = bacc.Bacc()
    q_d = nc.dram_tensor("q", [PAIRS_PER_CORE, L, E], bf16, kind="ExternalInput")
    k_d = nc.dram_tensor("k", [PAIRS_PER_CORE, S, E], bf16, kind="ExternalInput")
    v_d = nc.dram_tensor("v", [PAIRS_PER_CORE, S, D], bf16, kind="ExternalInput")
    m_d = nc.dram_tensor("masks", [128, _NDIST * WTOT], bf16, kind="ExternalInput")
    i_d = nc.dram_tensor("ident", [65, 65], f32, kind="ExternalInput")
    o_d = nc.dram_tensor("out", [PAIRS_PER_CORE, L, D], f32, kind="ExternalOutput")

    with tile.TileContext(nc) as tc:
        with (
            tc.tile_pool(name="const", bufs=1) as constp,
            tc.tile_pool(name="io", bufs=2) as iop,
            tc.tile_pool(name="sc", bufs=3) as scp,
            tc.tile_pool(name="ps", bufs=2, space=bass.MemorySpace.PSUM) as psp,
            tc.tile_pool(name="ot", bufs=1, space=bass.MemorySpace.PSUM) as otp,
        ):
            masks = constp.tile([128, _NDIST * WTOT], bf16)
            nc.sync.dma_start(masks[:], m_d[:])
            ident = constp.tile([65, 65], f32)
            nc.sync.dma_start(ident[:], i_d[:])
            zc = constp.tile([1, 65], bf16)
            nc.vector.memset(zc[:], 0.0)
            zr = constp.tile([1, 512], bf16)
            nc.vector.memset(zr[:], 0.0)

            for hh in range(PAIRS_PER_CORE):
                qt = iop.tile([64, QTW], bf16, tag="qt")
                nc.vector.memset(qt[:, L:QTW], 0.0)
                nc.sync.dma_start_transpose(qt[:, 0:L], q_d[hh])
                kt = iop.tile([64, S], bf16, tag="kt")
                nc.sync.dma_start_transpose(kt[:], k_d[hh])
                # V chunks with a ones column: [128, CH, 65]
                va = iop.tile([128, CH, 65], bf16, tag="va")
                nc.sync.dma_start(
                    va[:, :, 0:64],
                    v_d[hh].rearrange("(c p) e -> p c e", p=128),
                )
                nc.vector.memset(va[:, :, 64:65], 1.0)

                oT = otp.tile([65, S], f32, tag="oT")
                for kk in range(4):
                    nc.tensor.matmul(
                        oT[:, 512 * kk : 512 * (kk + 1)],
                        zc[:],
                        zr[:],
                        start=True,
                        stop=False,
                        skip_group_check=True,
                    )

                for c in range(CH):
                    j0 = 128 * c
                    ktc = kt[:, j0 : j0 + 128]
                    psA = psp.tile([128, WIN_A], f32, tag="psA")
                    psB = psp.tile([128, WIN_B], f32, tag="psB")
                    nc.tensor.matmul(
                        psA[:, 0:WB], ktc, qt[:, j0 : j0 + WB],
                        start=True, stop=True,
                    )
                    nc.tensor.matmul(
                        psA[:, WB:WIN_A], ktc,
                        qt[:, j0 + FARS[0] : j0 + FARS[0] + 128],
                        start=True, stop=True,
                    )
                    nc.tensor.matmul(
                        psB[:, 0:128], ktc,
                        qt[:, j0 + FARS[1] : j0 + FARS[1] + 128],
                        start=True, stop=True,
                    )
                    nc.tensor.matmul(
                        psB[:, 128:256], ktc,
                        qt[:, j0 + FARS[2] : j0 + FARS[2] + 128],
                        start=True, stop=True,
                    )
                    pA = scp.tile([128, WIN_A], bf16, tag="pA")
                    pB = scp.tile([128, WIN_B], bf16, tag="pB")
                    nc.scalar.activation(pA[:], psA[:], AF.Exp, scale=SCALE)
                    nc.scalar.activation(pB[:], psB[:], AF.Exp, scale=SCALE)
                    mo = _MASK_IDX[c] * WTOT
                    nc.vector.tensor_mul(pA[:], pA[:], masks[:, mo : mo + WIN_A])
                    nc.vector.tensor_mul(
                        pB[:], pB[:], masks[:, mo + WIN_A : mo + WTOT]
                    )
                    vac = va[:, c, :]
                    for tile_id, soff, dst, w, stop in _pv_pieces(c):
                        src = pA if tile_id == "A" else pB
                        nc.tensor.matmul(
                            oT[:, dst : dst + w],
                            vac,
                            src[:, soff : soff + w],
                            start=False,
                            stop=stop,
                            skip_group_check=True,
                        )

                # epilogue
                ots = iop.tile([65, S], f32, tag="ots")
                for kk in range(4):
                    nc.scalar.copy(
                        ots[:, 512 * kk : 512 * (kk + 1)],
                        oT[:, 512 * kk : 512 * (kk + 1)],
                    )
                for t in range(CH):
                    tp = psp.tile([128, 65], f32, tag="psA")
                    nc.tensor.transpose(tp[:], ots[:, 128 * t : 128 * t + 128], ident[:])
                    rz = scp.tile([128, 1], f32, tag="rz")
                    nc.vector.reciprocal(rz[:], tp[:, 64:65])
                    of = scp.tile([128, 64], f32, tag="of")
                    nc.scalar.mul(of[:], tp[:, 0:64], rz[:])
                    nc.sync.dma_start(o_d[hh, 128 * t : 128 * t + 128, :], of[:])

    nc.finalize()
    return nc


def _get_nc():
    if "nc" not in _CACHE:
        _CACHE["nc"] = _build_nc()
    return _CACHE["nc"]


# ---------------------------------------------------------------- entrypoint
def kernel(queries, keys, values, attention_mask=None, trace=False):
    from concourse.bass_utils import run_bass_kernel_spmd

    q = np.asarray(queries, dtype=np.float32)
    k = np.asarray(keys, dtype=np.float32)
    v = np.asarray(values, dtype=np.float32)

    # [B, L, H, E] -> [B*H, L, E]
    qp = np.ascontiguousarray(q.transpose(0, 2, 1, 3)).reshape(B * H, L, E)
    kp = np.ascontiguousarray(k.transpose(0, 2, 1, 3)).reshape(B * H, S, E)
    vp = np.ascontiguousarray(v.transpose(0, 2, 1, 3)).reshape(B * H, S, D)
    qb = qp.astype(ml_dtypes.bfloat16)
    kb = kp.astype(ml_dtypes.bfloat16)
    vb = vp.astype(ml_dtypes.bfloat16)
    eye = np.eye(65, dtype=np.float32)

    in_maps = []
    for m in range(NC_CORES):
        s0 = PAIRS_PER_CORE * m
        in_maps.append(
            {
                "q": np.ascontiguousarray(qb[s0 : s0 + PAIRS_PER_CORE]),
                "k": np.ascontiguousarray(kb[s0 : s0 + PAIRS_PER_CORE]),
                "v": np.ascontiguousarray(vb[s0 : s0 + PAIRS_PER_CORE]),
                "masks": _MASKS_NP,
                "ident": eye,
            }
        )

    nc = _get_nc()
    res = run_bass_kernel_spmd(
        nc, in_maps, core_ids=list(range(NC_CORES)), trace=trace
    )
    outs = np.stack([r["out"] for r in res.results])  # [8, 2, L, D]
    o = outs.reshape(B, H, L, D).transpose(0, 2, 1, 3)
    if trace:
        kernel.last_exec_time_ns = res.exec_time_ns
        kernel.last_results = res
    return np.ascontiguousarray(o.astype(np.float32))
